# revision 13
# baseline (speedup 1.0000x reference)
"""DefectNet (DGCNN-style point net) on 8 TRN2 NeuronCores via Bass/Tile.

Sharding: 1024 query points per core. kNN scores via a bf16-split PE matmul
(fp32-grade), chunked top-16 with DVE max8/max_index/match_replace + a DRAM
round-trip for candidate chunks. EdgeConv layers as gather-max over per-point
tables (max commutes with relu + per-query affine). Attention via the low-rank
trick (logits u_i.x_j with u = x @ (Wq Wk^T)/sqrt(D); y = (sum_k a x_j) @ Wv),
so only raw features are gathered. TransitionDown folded into one fused table.
Tables all-gathered between layers with on-device collectives.
"""
import sys
sys.path.insert(0, "/opt/trn_rl_repo")

import numpy as np
import concourse.bass as bass
import concourse.mybir as mybir
import concourse.tile as tile
from concourse import bacc
from concourse.bass_utils import run_bass_kernel_spmd
from concourse.masks import make_identity

F32 = mybir.dt.float32
F16 = mybir.dt.float16
BF16 = mybir.dt.bfloat16
I16 = mybir.dt.int16
U32 = mybir.dt.uint32
NPF16 = np.float16
NPBF = mybir.dt.np(BF16)

N = 8192
K = 16
NCORES = 8
NQ = N // NCORES          # 1024 queries per core
QB = NQ // 128            # 8 query blocks per core
MQ = NQ // 4              # 256 TD queries per core
TDQB = MQ // 128          # 2 TD query blocks
NCAND = 24                # candidate chunks (of 64) per query
NEG = -1e30

_cache = {}


def _split3(x):
    h = x.astype(NPBF).astype(np.float32)
    m = (x - h).astype(NPBF).astype(np.float32)
    l = (x - h - m).astype(NPBF).astype(np.float32)
    return h, m, l


def _build_program():
    nc = bacc.Bacc("TRN2", target_bir_lowering=False, debug=False,
                   num_devices=NCORES)

    def inp(name, shape, dt):
        return nc.declare_dram_parameter(name, list(shape), dt, isOutput=False)

    # ---- inputs (per-core tensors prepared on host) ----
    rhs_s = inp("rhs_s", [21, N], BF16)          # split p-side score rows
    lhsT_s = inp("lhsT_s", [21, NQ], BF16)       # split q-side (my queries)
    pT4 = inp("pT4", [4, NQ], F32)               # x,y,z,ones of my queries
    A1c = inp("A1c", [4, 64], F32)               # [Wc1f; bc1f]
    W1t4 = inp("W1t4", [4, 128], F32)            # [Wt1f pad to 128 cols; 0]
    A2c = inp("A2c", [65, 128], F16)             # [Wc2f; bc2f]
    W2tf = inp("W2tf", [64, 128], F16)
    G1 = inp("G1", [128, 128], F16)
    G2 = inp("G2", [256, 256], F16)
    Wv1f = inp("Wv1f", [128, 256], F16)
    ba1r = inp("ba1r", [1, 256], F16)
    Wv2f = inp("Wv2f", [256, 512], F16)
    ba2r = inp("ba2r", [1, 512], F16)
    Wfeatf = inp("Wfeatf", [512, 512], F16)      # TD feature part (x sd)
    Wxyzf4 = inp("Wxyzf4", [4, 512], F32)        # [Wxyz*sd; 0]
    Atd = inp("Atd", [4, 512], F32)              # [-Wxyz*sd; bd]
    Wu1f = inp("Wu1f", [512, 256], F16)
    Wu2f = inp("Wu2f", [512, 256], F32)
    Wc1f = inp("Wc1f", [256, 128], F16)
    Wc2c = inp("Wc2c", [128, 6], F16)
    bc2T = inp("bc2T", [6, 1], F32)
    bu_all = inp("bu_all", [256, 1], F32)        # bu1 + bu2 per channel
    bcfT = inp("bcfT", [128, 1], F32)            # classifier bias (transposed)
    W1c = inp("W1c", [128, 16], F32)             # idx wrap consts
    MC8 = inp("MC8", [128, 8], F32)
    Rrep = inp("Rrep", [16, 128], F32)
    iota24 = inp("iota24", [128, NCAND], F32)
    qrow = inp("qrow", [128, 1], F32)            # partition*128

    out_t = nc.declare_dram_parameter("outT", [6, MQ], F32, isOutput=True)
    dbg = nc.declare_dram_parameter("dbg", [128, QB, 16], F32, isOutput=True)

    # ---- internal DRAM ----
    s_dram = nc.dram_tensor("s_dram", [QB, 128 * 128, 64], F32)  # score chunks
    idx_dram = nc.dram_tensor("idx_dram", [NQ, 16], F32)
    ag1_in = nc.dram_tensor("ag1_in", [NQ, 128], F16)
    ag1_out = nc.dram_tensor("ag1_out", [N, 128], F16, addr_space="Shared")
    ag2_in = nc.dram_tensor("ag2_in", [NQ, 128], F16)
    ag2_out = nc.dram_tensor("ag2_out", [N, 128], F16, addr_space="Shared")
    ag3_in = nc.dram_tensor("ag3_in", [NQ, 128], F16)
    ag3_out = nc.dram_tensor("ag3_out", [N, 128], F16, addr_space="Shared")
    ag4_in = nc.dram_tensor("ag4_in", [NQ, 256], F16)
    ag4_out = nc.dram_tensor("ag4_out", [N, 256], F16, addr_space="Shared")
    ag5_in = nc.dram_tensor("ag5_in", [NQ, 512], F16)
    ag5_out = nc.dram_tensor("ag5_out", [N, 512], F16, addr_space="Shared")
    ar_in = nc.dram_tensor("ar_in", [1, 512], F32)
    ar_out = nc.dram_tensor("ar_out", [1, 512], F32, addr_space="Shared")

    RG = [list(range(NCORES))]

    with tile.TileContext(nc) as tc:
        _emit(nc, tc, locals())
    nc.compile()
    return nc


def _emit(nc, tc, T):
    import contextlib
    ctx = contextlib.ExitStack()
    with ctx:
        pers = ctx.enter_context(tc.tile_pool(name="pers", bufs=1))
        big = ctx.enter_context(tc.tile_pool(name="big", bufs=1))
        gat = ctx.enter_context(tc.tile_pool(name="gat", bufs=2))
        tmp = ctx.enter_context(tc.tile_pool(name="tmp", bufs=2))
        one = ctx.enter_context(tc.tile_pool(name="one", bufs=1))
        psA = ctx.enter_context(tc.tile_pool(name="psA", bufs=2, space="PSUM"))
        psB = ctx.enter_context(tc.tile_pool(name="psB", bufs=2, space="PSUM"))
        psC = ctx.enter_context(tc.tile_pool(name="psC", bufs=2, space="PSUM"))
        psG = ctx.enter_context(tc.tile_pool(name="psG", bufs=1, space="PSUM"))
        _body(nc, tc, T, pers, big, gat, tmp, one, psA, psB, psC, psG)


def oT_dummy(nc, one):
    t = one.tile([6, 256], mybir.dt.float32, tag="oT")
    nc.vector.memset(t[:], 0.0)
    return t


def _body(nc, tc, T, pers, big, gat, tmp, one, psA, psB, psC, psG):
    import os
    AX = mybir.AxisListType.X
    OP = mybir.AluOpType
    ACTF = mybir.ActivationFunctionType

    # ---------- persistent SBUF ----------
    rhs_s = pers.tile([21, N], BF16)
    nc.sync.dma_start(out=rhs_s[:], in_=T["rhs_s"][:])
    lhsT_s = pers.tile([21, NQ], BF16)
    nc.sync.dma_start(out=lhsT_s[:], in_=T["lhsT_s"][:])
    pT4 = pers.tile([4, NQ], F32)
    nc.sync.dma_start(out=pT4[:], in_=T["pT4"][:])

    def load(name, shape, dt, rearr=None):
        t = pers.tile(shape, dt, tag=name)
        src = T[name][:] if rearr is None else T[name].rearrange(rearr, p=128)
        nc.sync.dma_start(out=t[:], in_=src)
        return t

    A1c = load("A1c", [4, 64], F32)
    W1t4 = load("W1t4", [4, 128], F32)
    A2c = load("A2c", [65, 128], F16)
    W2tf = load("W2tf", [64, 128], F16)
    G1 = load("G1", [128, 128], F16)
    G2 = load("G2", [128, 2, 256], F16, "(c p) n -> p c n")
    Wv1f = load("Wv1f", [128, 256], F16)
    ba1r = load("ba1r", [1, 256], F16)
    Wv2f = load("Wv2f", [128, 2, 512], F16, "(c p) n -> p c n")
    ba2r = load("ba2r", [1, 512], F16)
    Wfeatf = load("Wfeatf", [128, 4, 512], F16, "(c p) n -> p c n")
    Wxyzf4 = load("Wxyzf4", [4, 512], F32)
    Atd = load("Atd", [4, 512], F32)
    Wu1f = load("Wu1f", [128, 4, 256], F16, "(c p) n -> p c n")
    Wu2f = load("Wu2f", [128, 4, 256], F32, "(c p) n -> p c n")
    Wc1f = load("Wc1f", [128, 2, 128], F16, "(c p) n -> p c n")
    Wc2c = load("Wc2c", [128, 6], F16)
    bc2T = load("bc2T", [6, 1], F32)
    bu_all = load("bu_all", [128, 2], F32, "(c p) n -> p (c n)")
    bcfT = load("bcfT", [128, 1], F32)
    W1c = load("W1c", [128, 16], F32)
    MC8 = load("MC8", [128, 8], F32)
    Rrep = load("Rrep", [16, 128], F32)
    iota24 = load("iota24", [128, NCAND], F32)
    qrow = load("qrow", [128, 1], F32)

    idf32 = pers.tile([128, 128], F32)
    make_identity(nc, idf32[:])
    idf16 = pers.tile([128, 128], F16)
    make_identity(nc, idf16[:])
    ones1 = pers.tile([1, 128], F16)
    nc.vector.memset(ones1[:], 1.0)
    ones128 = pers.tile([128, 1], F32)
    nc.vector.memset(ones128[:], 1.0)

    x1T = pers.tile([65, NQ], F16)       # row 64 = ones
    nc.vector.memset(x1T[64:65, :], 1.0)
    x2T = pers.tile([128, NQ], F16)
    x3T = pers.tile([128, 2, NQ], F16)
    x4T = pers.tile([128, 4, NQ], F16)
    idxf_all = pers.tile([128, QB, 16], F32)
    idx16_all = pers.tile([128, QB, 128], I16)

    def transpose(src_ap, pr, fr, dt):
        """PE transpose of src_ap [pr, fr] -> psum tile [fr, pr]."""
        ps = psC.tile([fr, 128], dt, tag="tr")
        ident = idf16 if dt == F16 else idf32
        nc.tensor.transpose(out=ps[:, :pr], in_=src_ap, identity=ident[:pr, :pr])
        return ps

    def wrap_idx(src_f32, ncols, dst_i16):
        """Build wrapped+replicated dma_gather idx tile from [128, ncols] fp32
        row-ids: dst[16g+w, 8c+m] = src[16m+w, c]."""
        prod = tmp.tile([128, ncols, 8], F32, tag="wrp")
        nc.vector.tensor_tensor(
            out=prod[:], in0=src_f32.to_broadcast([128, ncols, 8]),
            in1=MC8[:].rearrange("p (o m) -> p o m", o=1).to_broadcast([128, ncols, 8]),
            op=OP.mult)
        ps1 = psC.tile([16, 8 * ncols], F32, tag="tr")
        nc.tensor.matmul(out=ps1[:], lhsT=W1c[:],
                         rhs=prod[:].rearrange("p c m -> p (c m)"),
                         start=True, stop=True)
        w_sb = tmp.tile([16, 8 * ncols], F32, tag="wr2")
        nc.vector.tensor_copy(out=w_sb[:], in_=ps1[:])
        ps2 = psC.tile([128, 8 * ncols], F32, tag="tr")
        nc.tensor.matmul(out=ps2[:], lhsT=Rrep[:], rhs=w_sb[:],
                         start=True, stop=True)
        nc.vector.tensor_copy(out=dst_i16, in_=ps2[:])

    REPEAT = int(os.environ.get("BASS_REPEAT", "1"))
    for _rep in range(REPEAT):
        _phases(nc, tc, T, pers, big, gat, tmp, one, psA, psB, psC, psG,
                locals())


def _phases(nc, tc, T, pers, big, gat, tmp, one, psA, psB, psC, psG, L):
    AX = mybir.AxisListType
    AX = AX.X
    OP = mybir.AluOpType
    ACTF = mybir.ActivationFunctionType
    import os
    for _k in ("rhs_s lhsT_s pT4 A1c W1t4 A2c W2tf G1 G2 Wv1f ba1r Wv2f ba2r "
               "Wfeatf Wxyzf4 Atd Wu1f Wu2f Wc1f Wc2c bc2T bu_all bcfT W1c MC8 "
               "Rrep iota24 qrow idf32 idf16 ones1 ones128 x1T x2T x3T x4T "
               "idxf_all idx16_all transpose wrap_idx").split():
        globals()["_tmpvar"] = None
    (rhs_s, lhsT_s, pT4, A1c, W1t4, A2c, W2tf, G1, G2, Wv1f, ba1r, Wv2f, ba2r,
     Wfeatf, Wxyzf4, Atd, Wu1f, Wu2f, Wc1f, Wc2c, bc2T, bu_all, bcfT, W1c, MC8,
     Rrep, iota24, qrow, idf32, idf16, ones1, ones128, x1T, x2T, x3T, x4T,
     idxf_all, idx16_all, transpose, wrap_idx) = (
        L["rhs_s"], L["lhsT_s"], L["pT4"], L["A1c"], L["W1t4"], L["A2c"],
        L["W2tf"], L["G1"], L["G2"], L["Wv1f"], L["ba1r"], L["Wv2f"],
        L["ba2r"], L["Wfeatf"], L["Wxyzf4"], L["Atd"], L["Wu1f"], L["Wu2f"],
        L["Wc1f"], L["Wc2c"], L["bc2T"], L["bu_all"], L["bcfT"], L["W1c"],
        L["MC8"], L["Rrep"], L["iota24"], L["qrow"], L["idf32"], L["idf16"],
        L["ones1"], L["ones128"], L["x1T"], L["x2T"], L["x3T"], L["x4T"],
        L["idxf_all"], L["idx16_all"], L["transpose"], L["wrap_idx"])

    # ---------- phase A: T1 shard table + AG1 ----------
    tab1 = pers.tile([128, QB, 128], F16)
    for qb in range(QB):
        ps = psB.tile([128, 128], F32, tag="b")
        nc.tensor.matmul(out=ps[:], lhsT=pT4[:, qb * 128:(qb + 1) * 128],
                         rhs=W1t4[:], start=True, stop=True)
        nc.scalar.copy(out=tab1[:, qb, :], in_=ps[:])
    nc.sync.dma_start(
        out=T["ag1_in"].rearrange("(a p) c -> p a c", p=128),
        in_=tab1[:])
    nc.gpsimd.collective_compute(
        "AllGather", OP.bypass, ins=[T["ag1_in"][:]], outs=[T["ag1_out"][:]],
        replica_groups=T["RG"])

    # ---------- phase B: kNN ----------
    for qb in range(QB):
        s_sb = big.tile([128, N], F32, tag="s_sb")
        m64 = tmp.tile([128, 128], F32, tag="m64")
        for jt in range(16):
            ps = psA.tile([128, 512], F32, tag="sc")
            nc.tensor.matmul(out=ps[:], lhsT=lhsT_s[:, qb * 128:(qb + 1) * 128],
                             rhs=rhs_s[:, jt * 512:(jt + 1) * 512],
                             start=True, stop=True)
            nc.scalar.copy(out=s_sb[:, jt * 512:(jt + 1) * 512], in_=ps[:])
        for h in range(2):
            nc.vector.tensor_reduce(
                out=m64[:, h * 64:(h + 1) * 64],
                in_=s_sb[:, h * 4096:(h + 1) * 4096].rearrange(
                    "p (c e) -> p c e", e=64),
                axis=AX, op=OP.max)
        nc.sync.dma_start(
            out=T["s_dram"][qb].rearrange("(q c) e -> q (c e)", q=128),
            in_=s_sb[:])
        # top-24 chunks
        cid_u = tmp.tile([128, NCAND], U32, tag="cid_u")
        for r in range(3):
            v8 = tmp.tile([128, 8], F32, tag="v8")
            nc.vector.max(out=v8[:], in_=m64[:])
            nc.vector.max_index(out=cid_u[:, r * 8:(r + 1) * 8], in_max=v8[:],
                                in_values=m64[:])
            nc.vector.match_replace(out=m64[:], in_to_replace=v8[:],
                                    in_values=m64[:], imm_value=NEG)
        cid_f = tmp.tile([128, NCAND], F32, tag="cid_f")
        nc.vector.tensor_copy(out=cid_f[:], in_=cid_u[:])
        crow = tmp.tile([128, NCAND], F32, tag="crow")
        nc.vector.tensor_scalar(out=crow[:], in0=cid_f[:], scalar1=qrow[:, 0:1],
                                scalar2=None, op0=OP.add)
        cidx16 = tmp.tile([128, 8 * NCAND], I16, tag="cidx16")
        wrap_idx(crow[:], NCAND, cidx16[:])
        cand = gat.tile([128, NCAND, 64], F32, tag="cand")
        nc.gpsimd.dma_gather(out_ap=cand[:], in_ap=T["s_dram"][qb][:],
                             idxs_ap=cidx16[:], num_idxs=128 * NCAND,
                             num_idxs_reg=128 * NCAND, elem_size=64,
                             single_packet=False)
        # top-16 among candidates
        pos_u = tmp.tile([128, 16], U32, tag="pos_u")
        cand_f = cand[:].rearrange("p c e -> p (c e)")
        for r in range(2):
            v8 = tmp.tile([128, 8], F32, tag="v8")
            nc.vector.max(out=v8[:], in_=cand_f)
            nc.vector.max_index(out=pos_u[:, r * 8:(r + 1) * 8], in_max=v8[:],
                                in_values=cand_f)
            if r == 0:
                nc.vector.match_replace(out=cand_f, in_to_replace=v8[:],
                                        in_values=cand_f, imm_value=NEG)
        # j = cid[pos//64]*64 + pos%64
        csl_u = tmp.tile([128, 16], U32, tag="csl_u")
        nc.vector.tensor_scalar(out=csl_u[:], in0=pos_u[:], scalar1=6,
                                scalar2=None, op0=OP.logical_shift_right)
        rem_u = tmp.tile([128, 16], U32, tag="rem_u")
        nc.vector.tensor_scalar(out=rem_u[:], in0=pos_u[:], scalar1=63,
                                scalar2=None, op0=OP.bitwise_and)
        csf = tmp.tile([128, 16], F32, tag="csf")
        nc.vector.tensor_copy(out=csf[:], in_=csl_u[:])
        remf = tmp.tile([128, 16], F32, tag="remf")
        nc.vector.tensor_copy(out=remf[:], in_=rem_u[:])
        eq = tmp.tile([128, NCAND, 16], F32, tag="eq")
        nc.vector.tensor_tensor(
            out=eq[:], in0=csf[:].rearrange("p (o k) -> p o k", o=1).to_broadcast([128, NCAND, 16]),
            in1=iota24[:].to_broadcast([128, NCAND, 16]), op=OP.is_equal)
        nc.vector.tensor_tensor(
            out=eq[:], in0=eq[:],
            in1=cid_f[:].to_broadcast([128, NCAND, 16]), op=OP.mult)
        csel = tmp.tile([128, 16], F32, tag="csel")
        nc.vector.tensor_reduce(
            out=csel[:], in_=eq[:].rearrange("p c k -> p k c"),
            axis=AX, op=OP.add)
        nc.vector.tensor_scalar(out=csel[:], in0=csel[:], scalar1=64.0,
                                scalar2=None, op0=OP.mult)
        nc.vector.tensor_tensor(out=idxf_all[:, qb, :], in0=csel[:],
                                in1=remf[:], op=OP.add)
        nc.sync.dma_start(out=T["idx_dram"][qb * 128:(qb + 1) * 128, :],
                            in_=idxf_all[:, qb, :])
        wrap_idx(idxf_all[:, qb, :], 16, idx16_all[:, qb, :])

    import os
    nc.sync.dma_start(out=T["dbg"][:], in_=idxf_all[:])
    if os.environ.get("PHASES", "ALL") == "B":
        nc.sync.dma_start(out=T["out_t"][:], in_=oT_dummy(nc, one)[:])
        return

    # ---------- generic edge-conv stage ----------
    def layer_gathers(table_ap, C, qb_per):
        """Consolidated gathers: QB//qb_per dma_gathers over qb groups."""
        tiles = []
        for h in range(QB // qb_per):
            gb = gat.tile([128, qb_per * 16, C], F16, tag="g")
            nc.gpsimd.dma_gather(
                out_ap=gb[:], in_ap=table_ap,
                idxs_ap=idx16_all[:, h * qb_per:(h + 1) * qb_per, :].rearrange(
                    "p a c -> p (a c)"),
                num_idxs=qb_per * 2048, num_idxs_reg=qb_per * 2048,
                elem_size=C, single_packet=False)
            tiles.append(gb)

        def view(qb):
            return tiles[qb // qb_per][
                :, (qb % qb_per) * 16:((qb % qb_per) + 1) * 16, :]
        return view

    # ---------- phase C: EdgeConv1 (3 -> 64) ----------
    gv1 = layer_gathers(T["ag1_out"][:], 128, 4)
    for qb in range(QB):
        g = gv1(qb)
        mx = tmp.tile([128, 64], F32, tag="mx1")
        nc.vector.tensor_reduce(
            out=mx[:], in_=g[:, :, 0:64].rearrange("p k c -> p c k"),
            axis=AX, op=OP.max)
        ps = psB.tile([128, 64], F32, tag="b")
        nc.tensor.matmul(out=ps[:], lhsT=pT4[:, qb * 128:(qb + 1) * 128],
                         rhs=A1c[:], start=True, stop=True)
        nc.vector.tensor_tensor(out=mx[:], in0=mx[:], in1=ps[:], op=OP.add)
        x1q = tmp.tile([128, 64], F16, tag="x1q")
        nc.vector.tensor_scalar(out=x1q[:], in0=mx[:], scalar1=0.0,
                                scalar2=None, op0=OP.max)
        tr = transpose(x1q[:], 128, 64, F16)
        nc.vector.tensor_copy(out=x1T[0:64, qb * 128:(qb + 1) * 128],
                              in_=tr[:, :128])

    # ---------- phase D: T2 shard + AG2 ----------
    tab2 = pers.tile([128, QB, 128], F16)
    for qb in range(QB):
        ps = psB.tile([128, 128], F32, tag="b")
        nc.tensor.matmul(out=ps[:], lhsT=x1T[0:64, qb * 128:(qb + 1) * 128],
                         rhs=W2tf[:], start=True, stop=True)
        nc.scalar.copy(out=tab2[:, qb, :], in_=ps[:])
    nc.sync.dma_start(
        out=T["ag2_in"].rearrange("(a p) c -> p a c", p=128), in_=tab2[:])
    nc.gpsimd.collective_compute(
        "AllGather", OP.bypass, ins=[T["ag2_in"][:]], outs=[T["ag2_out"][:]],
        replica_groups=T["RG"])

    # ---------- phase E: EdgeConv2 (64 -> 128) + x2 AG ----------
    gv2 = layer_gathers(T["ag2_out"][:], 128, 4)
    for qb in range(QB):
        g = gv2(qb)
        mx = tmp.tile([128, 128], F32, tag="mx2")
        nc.vector.tensor_reduce(
            out=mx[:], in_=g[:].rearrange("p k c -> p c k"), axis=AX, op=OP.max)
        ps = psB.tile([128, 128], F32, tag="b")
        nc.tensor.matmul(out=ps[:], lhsT=x1T[:, qb * 128:(qb + 1) * 128],
                         rhs=A2c[:], start=True, stop=True)
        nc.vector.tensor_tensor(out=mx[:], in0=mx[:], in1=ps[:], op=OP.add)
        x2q = tmp.tile([128, 128], F16, tag="x2q")
        nc.vector.tensor_scalar(out=x2q[:], in0=mx[:], scalar1=0.0,
                                scalar2=None, op0=OP.max)
        tr = transpose(x2q[:], 128, 128, F16)
        nc.vector.tensor_copy(out=x2T[:, qb * 128:(qb + 1) * 128], in_=tr[:])
        nc.sync.dma_start(out=T["ag3_in"][qb * 128:(qb + 1) * 128, :],
                            in_=x2q[:])
    nc.gpsimd.collective_compute(
        "AllGather", OP.bypass, ins=[T["ag3_in"][:]], outs=[T["ag3_out"][:]],
        replica_groups=T["RG"])

    # ---------- attention stage ----------
    def attention(qb, g, D, u_tile, zT_out):
        """Gathered x_j rows g [128,16,D], logits via ttr with u, softmax, z."""
        l = tmp.tile([128, 16], F32, tag="l")
        prod = one.tile([128, 16, D], F16, tag="prod")
        nc.vector.tensor_tensor(
            out=prod[:], in0=g[:],
            in1=u_tile.rearrange("p (o c) -> p o c", o=1).to_broadcast([128, 16, D]),
            op=OP.mult)
        nc.vector.tensor_reduce(out=l[:], in_=prod[:], axis=AX, op=OP.add)
        mx = tmp.tile([128, 1], F32, tag="lm")
        nc.vector.tensor_reduce(out=mx[:], in_=l[:], axis=AX, op=OP.max)
        nc.vector.tensor_scalar(out=l[:], in0=l[:], scalar1=mx[:, 0:1],
                                scalar2=None, op0=OP.subtract)
        a = tmp.tile([128, 16], F32, tag="a")
        nc.scalar.activation(out=a[:], in_=l[:], func=ACTF.Exp)
        ssum = tmp.tile([128, 1], F32, tag="ssum")
        nc.vector.tensor_reduce(out=ssum[:], in_=a[:], axis=AX, op=OP.add)
        rcp = tmp.tile([128, 1], F32, tag="rcp")
        nc.vector.reciprocal(out=rcp[:], in_=ssum[:])
        nc.vector.tensor_scalar(out=a[:], in0=a[:], scalar1=rcp[:, 0:1],
                                scalar2=None, op0=OP.mult)
        for k in range(16):
            nc.scalar.activation(out=g[:, k, :], in_=g[:, k, :],
                                 func=ACTF.Copy, scale=a[:, k:k + 1])
        nc.vector.tensor_tensor(out=g[:, 0:8, :], in0=g[:, 0:8, :],
                                in1=g[:, 8:16, :], op=OP.add)
        nc.vector.tensor_tensor(out=g[:, 0:4, :], in0=g[:, 0:4, :],
                                in1=g[:, 4:8, :], op=OP.add)
        nc.vector.tensor_tensor(out=g[:, 0:2, :], in0=g[:, 0:2, :],
                                in1=g[:, 2:4, :], op=OP.add)
        zf = tmp.tile([128, D], F16, tag=f"zf{D}")
        nc.vector.tensor_tensor(out=zf[:], in0=g[:, 0, :],
                                in1=g[:, 1, :], op=OP.add)
        for c in range(D // 128):
            tr = transpose(zf[:, c * 128:(c + 1) * 128], 128, 128, F16)
            nc.vector.tensor_copy(out=zT_out[:, c, :], in_=tr[:])

    if os.environ.get("PHASES", "ALL") == "E":
        nc.sync.dma_start(out=T["out_t"][:], in_=oT_dummy(nc, one)[:])
        return

    # ---------- phase F: attention 1 (128 -> 256) + x3/AG4 ----------
    gv3 = layer_gathers(T["ag3_out"][:], 128, 4)
    for qb in range(QB):
        psu = psB.tile([128, 128], F32, tag="b")
        nc.tensor.matmul(out=psu[:], lhsT=x2T[:, qb * 128:(qb + 1) * 128],
                         rhs=G1[:], start=True, stop=True)
        u1 = tmp.tile([128, 128], F16, tag="u1")
        nc.scalar.copy(out=u1[:], in_=psu[:])
        z1T = tmp.tile([128, 1, 128], F16, tag="z1T")
        attention(qb, gv3(qb), 128, u1[:], z1T[:])
        psy = psB.tile([128, 256], F32, tag="b")
        nc.tensor.matmul(out=psy[:], lhsT=z1T[:, 0, :], rhs=Wv1f[:],
                         start=True, stop=False)
        nc.tensor.matmul(out=psy[:], lhsT=ones1[:], rhs=ba1r[:],
                         start=False, stop=True)
        x3q = tmp.tile([128, 256], F16, tag="x3q")
        nc.vector.tensor_scalar(out=x3q[:], in0=psy[:], scalar1=0.0,
                                scalar2=None, op0=OP.max)
        for c in range(2):
            tr = transpose(x3q[:, c * 128:(c + 1) * 128], 128, 128, F16)
            nc.vector.tensor_copy(out=x3T[:, c, qb * 128:(qb + 1) * 128],
                                  in_=tr[:])
        nc.sync.dma_start(out=T["ag4_in"][qb * 128:(qb + 1) * 128, :],
                            in_=x3q[:])
    nc.gpsimd.collective_compute(
        "AllGather", OP.bypass, ins=[T["ag4_in"][:]], outs=[T["ag4_out"][:]],
        replica_groups=T["RG"])

    if os.environ.get("PHASES", "ALL") == "F":
        nc.sync.dma_start(out=T["out_t"][:], in_=oT_dummy(nc, one)[:])
        return

    # ---------- phase G: attention 2 (256 -> 512) ----------
    gv4 = layer_gathers(T["ag4_out"][:], 256, 2)
    for qb in range(QB):
        psu = psB.tile([128, 256], F32, tag="b")
        for c in range(2):
            nc.tensor.matmul(out=psu[:], lhsT=x3T[:, c, qb * 128:(qb + 1) * 128],
                             rhs=G2[:, c, :], start=(c == 0), stop=(c == 1))
        u2 = tmp.tile([128, 256], F16, tag="u2")
        nc.scalar.copy(out=u2[:], in_=psu[:])
        z2T = tmp.tile([128, 2, 128], F16, tag="z2T")
        attention(qb, gv4(qb), 256, u2[:], z2T[:])
        psy = psB.tile([128, 512], F32, tag="b")
        for c in range(2):
            nc.tensor.matmul(out=psy[:], lhsT=z2T[:, c, :], rhs=Wv2f[:, c, :],
                             start=(c == 0), stop=False)
        nc.tensor.matmul(out=psy[:], lhsT=ones1[:], rhs=ba2r[:],
                         start=False, stop=True)
        x4q = tmp.tile([128, 512], F16, tag="x4q")
        nc.vector.tensor_scalar(out=x4q[:], in0=psy[:], scalar1=0.0,
                                scalar2=None, op0=OP.max)
        for c in range(4):
            tr = transpose(x4q[:, c * 128:(c + 1) * 128], 128, 128, F16)
            nc.vector.tensor_copy(out=x4T[:, c, qb * 128:(qb + 1) * 128],
                                  in_=tr[:])

    # ---------- phase H: TD table + AG5 ----------
    tabT = pers.tile([128, QB, 512], F16)
    for qb in range(QB):
        ps = psB.tile([128, 512], F32, tag="b")
        for c in range(4):
            nc.tensor.matmul(out=ps[:], lhsT=x4T[:, c, qb * 128:(qb + 1) * 128],
                             rhs=Wfeatf[:, c, :], start=(c == 0), stop=False)
        nc.tensor.matmul(out=ps[:], lhsT=pT4[:, qb * 128:(qb + 1) * 128],
                         rhs=Wxyzf4[:], start=False, stop=True)
        nc.scalar.copy(out=tabT[:, qb, :], in_=ps[:])
    nc.sync.dma_start(
        out=T["ag5_in"].rearrange("(a p) c -> p a c", p=128), in_=tabT[:])
    nc.gpsimd.collective_compute(
        "AllGather", OP.bypass, ins=[T["ag5_in"][:]], outs=[T["ag5_out"][:]],
        replica_groups=T["RG"])

    if os.environ.get("PHASES", "ALL") == "H":
        nc.sync.dma_start(out=T["out_t"][:], in_=oT_dummy(nc, one)[:])
        return

    # ---------- phase I: TransitionDown ----------
    xdT = pers.tile([128, 4, MQ], F16)
    psg = psG.tile([1, 512], F32, tag="gsum")
    idxd16_all = one.tile([128, TDQB, 128], I16, tag="idxd16")
    for t in range(TDQB):
        idxd = one.tile([128, 16], F32, tag=f"idxd{t}")
        nc.sync.dma_start(
            out=idxd[:],
            in_=T["idx_dram"][t * 512:(t + 1) * 512:4, :])
        wrap_idx(idxd[:], 16, idxd16_all[:, t, :])
    gtds = []
    for t in range(TDQB):
        gtd = gat.tile([128, 16, 512], F16, tag="g")
        nc.gpsimd.dma_gather(out_ap=gtd[:], in_ap=T["ag5_out"][:],
                             idxs_ap=idxd16_all[:, t, :], num_idxs=2048,
                             num_idxs_reg=2048, elem_size=512,
                             single_packet=False)
        gtds.append(gtd)
    for t in range(TDQB):
        g = gtds[t]
        mx = one.tile([128, 512], F32, tag="mxtd")
        nc.vector.tensor_reduce(
            out=mx[:], in_=g[:].rearrange("p k c -> p c k"), axis=AX, op=OP.max)
        ps = psB.tile([128, 512], F32, tag="b")
        nc.tensor.matmul(out=ps[:], lhsT=pT4[:, t * 512::4][:, :128],
                         rhs=Atd[:], start=True, stop=True)
        xd = one.tile([128, 512], F32, tag="xd")
        nc.vector.tensor_tensor(out=xd[:], in0=mx[:], in1=ps[:], op=OP.add)
        nc.vector.tensor_scalar(out=xd[:], in0=xd[:], scalar1=0.0,
                                scalar2=None, op0=OP.max)
        nc.tensor.matmul(out=psg[:], lhsT=ones128[:], rhs=xd[:],
                         start=(t == 0), stop=(t == TDQB - 1))
        xdf = one.tile([128, 512], F16, tag="xdf")
        nc.vector.tensor_copy(out=xdf[:], in_=xd[:])
        for c in range(4):
            tr = transpose(xdf[:, c * 128:(c + 1) * 128], 128, 128, F16)
            nc.vector.tensor_copy(out=xdT[:, c, t * 128:(t + 1) * 128],
                                  in_=tr[:])
    gsum = one.tile([1, 512], F32, tag="gsumsb")
    nc.vector.tensor_copy(out=gsum[:], in_=psg[:])
    nc.sync.dma_start(out=T["ar_in"][:], in_=gsum[:])
    nc.gpsimd.collective_compute(
        "AllReduce", OP.add, ins=[T["ar_in"][:]], outs=[T["ar_out"][:]],
        replica_groups=T["RG"])

    # ---------- phase J: TransitionUp + classifier ----------
    gs = one.tile([1, 512], F32, tag="gs")
    nc.sync.dma_start(out=gs[:], in_=T["ar_out"][:])
    gm = one.tile([1, 512], F32, tag="gm")
    nc.scalar.mul(out=gm[:], in_=gs[:], mul=1.0 / 2048.0)
    gmT = one.tile([128, 4], F32, tag="gmT")
    for c in range(4):
        tr = transpose(gm[0:1, c * 128:(c + 1) * 128], 1, 128, F32)
        nc.vector.tensor_copy(out=gmT[:, c:c + 1], in_=tr[:, 0:1])
    gbT = one.tile([128, 2], F32, tag="gbT")
    for cc in range(2):
        ps = psC.tile([128, 1], F32, tag="tr")
        for c in range(4):
            nc.tensor.matmul(out=ps[:], lhsT=Wu2f[:, c, cc * 128:(cc + 1) * 128],
                             rhs=gmT[:, c:c + 1], start=(c == 0), stop=(c == 3))
        nc.vector.tensor_copy(out=gbT[:, cc:cc + 1], in_=ps[:])
    nc.vector.tensor_tensor(out=gbT[:], in0=gbT[:], in1=bu_all[:], op=OP.add)

    xuT = one.tile([128, 2, MQ], F16, tag="xuT")
    for cc in range(2):
        ps = psB.tile([128, MQ], F32, tag="b")
        for c in range(4):
            nc.tensor.matmul(out=ps[:], lhsT=Wu1f[:, c, cc * 128:(cc + 1) * 128],
                             rhs=xdT[:, c, :], start=(c == 0), stop=(c == 3))
        nc.scalar.activation(out=xuT[:, cc, :], in_=ps[:], func=ACTF.Relu,
                             bias=gbT[:, cc:cc + 1], scale=1.0)
    psh = psB.tile([128, MQ], F32, tag="b")
    for c in range(2):
        nc.tensor.matmul(out=psh[:], lhsT=Wc1f[:, c, :], rhs=xuT[:, c, :],
                         start=(c == 0), stop=(c == 1))
    hT = one.tile([128, MQ], F16, tag="hT")
    nc.scalar.activation(out=hT[:], in_=psh[:], func=ACTF.Relu,
                         bias=bcfT[:, 0:1], scale=1.0)
    pso = psC.tile([6, MQ], F32, tag="tr")
    nc.tensor.matmul(out=pso[:], lhsT=Wc2c[:], rhs=hT[:], start=True, stop=True)
    oT = one.tile([6, MQ], F32, tag="oT")
    nc.scalar.add(out=oT[:], in_=pso[:], add=bc2T[:, 0:1])
    nc.sync.dma_start(out=T["out_t"][:], in_=oT[:])


def _prepare_inputs(inputs):
    p = np.asarray(inputs["points"])[0].astype(np.float32)        # [N, 3]
    pp = (p.astype(np.float64) ** 2).sum(1).astype(np.float32)

    qh, qm, ql = _split3(2.0 * p.T)                   # [3, N]
    ph, pm_, pl = _split3(p.T)
    pph, ppm, ppl = _split3(-pp[None, :])
    onesN = np.ones((1, N), np.float32)
    lhsTs_full = np.concatenate([qh, qh, qh, qm, qm, ql, onesN, onesN, onesN],
                                0).astype(NPBF)       # [21, N]
    rhss = np.concatenate([ph, pm_, pl, ph, pm_, ph, pph, ppm, ppl],
                          0).astype(NPBF)             # [21, N]
    pT4_full = np.concatenate([p.T, onesN], 0).astype(np.float32)  # [4, N]

    f32 = np.float32
    We1, se1, be1 = f32(inputs["We1"]), f32(inputs["se1"]), f32(inputs["be1"])
    We2, se2, be2 = f32(inputs["We2"]), f32(inputs["se2"]), f32(inputs["be2"])
    Wq1, Wk1, Wv1 = f32(inputs["Wq1"]), f32(inputs["Wk1"]), f32(inputs["Wv1"])
    sa1, ba1 = f32(inputs["sa1"]), f32(inputs["ba1"])
    Wq2, Wk2, Wv2 = f32(inputs["Wq2"]), f32(inputs["Wk2"]), f32(inputs["Wv2"])
    sa2, ba2 = f32(inputs["sa2"]), f32(inputs["ba2"])
    Wtd, sd, bd = f32(inputs["Wtd"]), f32(inputs["sd"]), f32(inputs["bd"])
    Wu1, su1, bu1 = f32(inputs["Wu1"]), f32(inputs["su1"]), f32(inputs["bu1"])
    Wu2, su2, bu2 = f32(inputs["Wu2"]), f32(inputs["su2"]), f32(inputs["bu2"])
    Wc1, bc1 = f32(inputs["Wc1"]), f32(inputs["bc1"])
    sc, bc = f32(inputs["sc"]), f32(inputs["bc"])
    Wc2, bc2 = f32(inputs["Wc2"]), f32(inputs["bc2"])

    A1c = np.concatenate([(We1[:3] - We1[3:]) * se1, (be1)[None, :]], 0)
    W1t4 = np.zeros((4, 128), np.float32)
    W1t4[:3, :64] = We1[3:] * se1
    A2c = np.concatenate([(We2[:64] - We2[64:]) * se2, be2[None, :]],
                         0).astype(NPF16)
    W2tf = (We2[64:] * se2).astype(NPF16)
    G1 = (Wq1 @ Wk1.T / np.sqrt(256.0)).astype(NPF16)
    G2 = (Wq2 @ Wk2.T / np.sqrt(512.0)).astype(NPF16)
    Wv1f = (Wv1 * sa1).astype(NPF16)
    Wv2f = (Wv2 * sa2).astype(NPF16)
    Wxyz = Wtd[:3] * sd
    Wxyzf4 = np.concatenate([Wxyz, np.zeros((1, 512), np.float32)], 0)
    Atd = np.concatenate([-Wxyz, bd[None, :]], 0)
    Wfeatf = (Wtd[3:] * sd).astype(NPF16)
    Wu1f = (Wu1 * su1).astype(NPF16)
    Wu2f = (Wu2 * su2).astype(np.float32)
    Wc1f = (Wc1 * sc).astype(NPF16)
    bcf = (bc1 * sc + bc).astype(np.float32)

    qi = np.arange(128)
    W1c = (qi[:, None] % 16 == np.arange(16)[None, :]).astype(np.float32)
    MC8 = (qi[:, None] // 16 == np.arange(8)[None, :]).astype(np.float32)
    Rrep = (np.arange(16)[:, None] == qi[None, :] % 16).astype(np.float32)
    iota24 = np.tile(np.arange(NCAND, dtype=np.float32)[None, :], (128, 1))
    qrow = (qi * 128).astype(np.float32)[:, None]

    common = dict(
        rhs_s=rhss, A1c=A1c, W1t4=W1t4, A2c=A2c, W2tf=W2tf, G1=G1, G2=G2,
        Wv1f=Wv1f, ba1r=(ba1[None, :]).astype(NPF16),
        Wv2f=Wv2f, ba2r=(ba2[None, :]).astype(NPF16),
        Wfeatf=Wfeatf, Wxyzf4=Wxyzf4, Atd=Atd,
        Wu1f=Wu1f, Wu2f=Wu2f, Wc1f=Wc1f,
        Wc2c=Wc2.astype(NPF16), bc2T=bc2.astype(np.float32)[:, None],
        bu_all=(bu1 + bu2).astype(np.float32)[:, None],
        bcfT=bcf[:, None],
        W1c=W1c, MC8=MC8, Rrep=Rrep, iota24=iota24, qrow=qrow,
    )
    in_maps = []
    for c in range(NCORES):
        m = dict(common)
        m["lhsT_s"] = np.ascontiguousarray(lhsTs_full[:, c * NQ:(c + 1) * NQ])
        m["pT4"] = np.ascontiguousarray(pT4_full[:, c * NQ:(c + 1) * NQ])
        in_maps.append(m)
    return in_maps


def kernel(**inputs):
    if "nc" not in _cache:
        _cache["nc"] = _build_program()
    nc = _cache["nc"]
    in_maps = _prepare_inputs(inputs)
    res = run_bass_kernel_spmd(nc, in_maps, core_ids=list(range(NCORES)))
    out = np.zeros((2048, 6), np.float32)
    for c in range(NCORES):
        out[c * MQ:(c + 1) * MQ, :] = res.results[c]["outT"].T
    _cache["dbg"] = [res.results[c]["dbg"] for c in range(NCORES)]
    _cache["res"] = res
    return out



# revision 15
# speedup vs baseline: 1.1556x; 1.1556x over previous
"""DefectNet (DGCNN-style point net) on 8 TRN2 NeuronCores via Bass/Tile.

Sharding: 1024 query points per core. kNN scores via a bf16-split PE matmul
(fp32-grade), chunked top-16 with DVE max8/max_index/match_replace + a DRAM
round-trip for candidate chunks. EdgeConv layers as gather-max over per-point
tables (max commutes with relu + per-query affine). Attention via the low-rank
trick (logits u_i.x_j with u = x @ (Wq Wk^T)/sqrt(D); y = (sum_k a x_j) @ Wv),
so only raw features are gathered. TransitionDown folded into one fused table.
Tables all-gathered between layers with on-device collectives.
"""
import sys
sys.path.insert(0, "/opt/trn_rl_repo")

import numpy as np
import concourse.bass as bass
import concourse.mybir as mybir
import concourse.tile as tile
from concourse import bacc
from concourse.bass_utils import run_bass_kernel_spmd
from concourse.masks import make_identity

F32 = mybir.dt.float32
F16 = mybir.dt.float16
BF16 = mybir.dt.bfloat16
I16 = mybir.dt.int16
U32 = mybir.dt.uint32
NPF16 = np.float16
NPBF = mybir.dt.np(BF16)

N = 8192
K = 16
NCORES = 8
NQ = N // NCORES          # 1024 queries per core
QB = NQ // 128            # 8 query blocks per core
MQ = NQ // 4              # 256 TD queries per core
TDQB = MQ // 128          # 2 TD query blocks
NCAND = 24                # candidate chunks (of 64) per query
NEG = -1e30

_cache = {}


def _split3(x):
    h = x.astype(NPBF).astype(np.float32)
    m = (x - h).astype(NPBF).astype(np.float32)
    l = (x - h - m).astype(NPBF).astype(np.float32)
    return h, m, l


def _build_program():
    nc = bacc.Bacc("TRN2", target_bir_lowering=False, debug=False,
                   num_devices=NCORES)

    def inp(name, shape, dt):
        return nc.declare_dram_parameter(name, list(shape), dt, isOutput=False)

    # ---- inputs (per-core tensors prepared on host) ----
    rhs_s = inp("rhs_s", [21, N], BF16)          # split p-side score rows
    lhsT_s = inp("lhsT_s", [21, NQ], BF16)       # split q-side (my queries)
    pT4 = inp("pT4", [4, NQ], F32)               # x,y,z,ones of my queries
    A1c = inp("A1c", [4, 64], F32)               # [Wc1f; bc1f]
    W1t4 = inp("W1t4", [4, 128], F32)            # [Wt1f pad to 128 cols; 0]
    A2c = inp("A2c", [65, 128], F16)             # [Wc2f; bc2f]
    W2tf = inp("W2tf", [64, 128], F16)
    G1 = inp("G1", [128, 128], F16)
    G2 = inp("G2", [256, 256], F16)
    Wv1f = inp("Wv1f", [128, 256], F16)
    ba1r = inp("ba1r", [1, 256], F16)
    Wv2f = inp("Wv2f", [256, 512], F16)
    ba2r = inp("ba2r", [1, 512], F16)
    Wfeatf = inp("Wfeatf", [512, 512], F16)      # TD feature part (x sd)
    Wxyzf4 = inp("Wxyzf4", [4, 512], F32)        # [Wxyz*sd; 0]
    Atd = inp("Atd", [4, 512], F32)              # [-Wxyz*sd; bd]
    Wu1f = inp("Wu1f", [512, 256], F16)
    Wu2f = inp("Wu2f", [512, 256], F32)
    Wc1f = inp("Wc1f", [256, 128], F16)
    Wc2c = inp("Wc2c", [128, 6], F16)
    bc2T = inp("bc2T", [6, 1], F32)
    bu_all = inp("bu_all", [256, 1], F32)        # bu1 + bu2 per channel
    bcfT = inp("bcfT", [128, 1], F32)            # classifier bias (transposed)
    W1c = inp("W1c", [128, 16], F32)             # idx wrap consts
    MC8 = inp("MC8", [128, 8], F32)
    Rrep = inp("Rrep", [16, 128], F32)
    iota24 = inp("iota24", [128, NCAND], F32)
    qrow = inp("qrow", [128, 1], F32)            # partition*128

    out_t = nc.declare_dram_parameter("outT", [6, MQ], F32, isOutput=True)
    dbg = nc.declare_dram_parameter("dbg", [128, QB, 16], F32, isOutput=True)

    # ---- internal DRAM ----
    s_dram = nc.dram_tensor("s_dram", [QB, 128 * 128, 64], F32)  # score chunks
    idx_dram = nc.dram_tensor("idx_dram", [NQ, 16], F32)
    ag1_in = nc.dram_tensor("ag1_in", [NQ, 128], F16)
    ag1_out = nc.dram_tensor("ag1_out", [N, 128], F16, addr_space="Shared")
    ag2_in = nc.dram_tensor("ag2_in", [NQ, 128], F16)
    ag2_out = nc.dram_tensor("ag2_out", [N, 128], F16, addr_space="Shared")
    ag3_in = nc.dram_tensor("ag3_in", [NQ, 128], F16)
    ag3_out = nc.dram_tensor("ag3_out", [N, 128], F16, addr_space="Shared")
    ag4_in = nc.dram_tensor("ag4_in", [NQ, 256], F16)
    ag4_out = nc.dram_tensor("ag4_out", [N, 256], F16, addr_space="Shared")
    ag5_in = nc.dram_tensor("ag5_in", [NQ, 512], F16)
    ag5_out = nc.dram_tensor("ag5_out", [N, 512], F16, addr_space="Shared")
    ar_in = nc.dram_tensor("ar_in", [1, 512], F32)
    ar_out = nc.dram_tensor("ar_out", [1, 512], F32, addr_space="Shared")

    RG = [list(range(NCORES))]

    with tile.TileContext(nc) as tc:
        _emit(nc, tc, locals())
    nc.compile()
    return nc


def _emit(nc, tc, T):
    import contextlib
    ctx = contextlib.ExitStack()
    with ctx:
        pers = ctx.enter_context(tc.tile_pool(name="pers", bufs=1))
        big = ctx.enter_context(tc.tile_pool(name="big", bufs=1))
        gat = ctx.enter_context(tc.tile_pool(name="gat", bufs=2))
        tmp = ctx.enter_context(tc.tile_pool(name="tmp", bufs=2))
        one = ctx.enter_context(tc.tile_pool(name="one", bufs=1))
        psA = ctx.enter_context(tc.tile_pool(name="psA", bufs=2, space="PSUM"))
        psB = ctx.enter_context(tc.tile_pool(name="psB", bufs=2, space="PSUM"))
        psC = ctx.enter_context(tc.tile_pool(name="psC", bufs=2, space="PSUM"))
        psG = ctx.enter_context(tc.tile_pool(name="psG", bufs=1, space="PSUM"))
        _body(nc, tc, T, pers, big, gat, tmp, one, psA, psB, psC, psG)


def oT_dummy(nc, one):
    t = one.tile([6, 256], mybir.dt.float32, tag="oT")
    nc.vector.memset(t[:], 0.0)
    return t


def _body(nc, tc, T, pers, big, gat, tmp, one, psA, psB, psC, psG):
    import os
    AX = mybir.AxisListType.X
    OP = mybir.AluOpType
    ACTF = mybir.ActivationFunctionType

    # ---------- persistent SBUF ----------
    rhs_s = pers.tile([21, N], BF16)
    nc.sync.dma_start(out=rhs_s[:], in_=T["rhs_s"][:])
    lhsT_s = pers.tile([21, NQ], BF16)
    nc.sync.dma_start(out=lhsT_s[:], in_=T["lhsT_s"][:])
    pT4 = pers.tile([4, NQ], F32)
    nc.sync.dma_start(out=pT4[:], in_=T["pT4"][:])

    def load(name, shape, dt, rearr=None):
        t = pers.tile(shape, dt, tag=name)
        src = T[name][:] if rearr is None else T[name].rearrange(rearr, p=128)
        nc.sync.dma_start(out=t[:], in_=src)
        return t

    A1c = load("A1c", [4, 64], F32)
    W1t4 = load("W1t4", [4, 128], F32)
    A2c = load("A2c", [65, 128], F16)
    W2tf = load("W2tf", [64, 128], F16)
    G1 = load("G1", [128, 128], F16)
    G2 = load("G2", [128, 2, 256], F16, "(c p) n -> p c n")
    Wv1f = load("Wv1f", [128, 256], F16)
    ba1r = load("ba1r", [1, 256], F16)
    Wv2f = load("Wv2f", [128, 2, 512], F16, "(c p) n -> p c n")
    ba2r = load("ba2r", [1, 512], F16)
    Wfeatf = load("Wfeatf", [128, 4, 512], F16, "(c p) n -> p c n")
    Wxyzf4 = load("Wxyzf4", [4, 512], F32)
    Atd = load("Atd", [4, 512], F32)
    Wu1f = load("Wu1f", [128, 4, 256], F16, "(c p) n -> p c n")
    Wu2f = load("Wu2f", [128, 4, 256], F32, "(c p) n -> p c n")
    Wc1f = load("Wc1f", [128, 2, 128], F16, "(c p) n -> p c n")
    Wc2c = load("Wc2c", [128, 6], F16)
    bc2T = load("bc2T", [6, 1], F32)
    bu_all = load("bu_all", [128, 2], F32, "(c p) n -> p (c n)")
    bcfT = load("bcfT", [128, 1], F32)
    W1c = load("W1c", [128, 16], F32)
    MC8 = load("MC8", [128, 8], F32)
    Rrep = load("Rrep", [16, 128], F32)
    iota24 = load("iota24", [128, NCAND], F32)
    qrow = load("qrow", [128, 1], F32)

    idf32 = pers.tile([128, 128], F32)
    make_identity(nc, idf32[:])
    idf16 = pers.tile([128, 128], F16)
    make_identity(nc, idf16[:])
    ones1 = pers.tile([1, 128], F16)
    nc.vector.memset(ones1[:], 1.0)
    ones128 = pers.tile([128, 1], F32)
    nc.vector.memset(ones128[:], 1.0)

    x1T = pers.tile([65, NQ], F16)       # row 64 = ones
    nc.vector.memset(x1T[64:65, :], 1.0)
    x2T = pers.tile([128, NQ], F16)
    x3T = pers.tile([128, 2, NQ], F16)
    x4T = pers.tile([128, 4, NQ], F16)
    idxf_all = pers.tile([128, QB, 16], F32)
    idx16_all = pers.tile([128, QB, 128], I16)

    def transpose(src_ap, pr, fr, dt):
        """PE transpose of src_ap [pr, fr] -> psum tile [fr, pr]."""
        ps = psC.tile([fr, 128], dt, tag="tr")
        ident = idf16 if dt == F16 else idf32
        nc.tensor.transpose(out=ps[:, :pr], in_=src_ap, identity=ident[:pr, :pr])
        return ps

    def wrap_idx(src_f32, ncols, dst_i16):
        """Build wrapped+replicated dma_gather idx tile from [128, ncols] fp32
        row-ids: dst[16g+w, 8c+m] = src[16m+w, c]."""
        prod = tmp.tile([128, ncols, 8], F32, tag="wrp")
        nc.vector.tensor_tensor(
            out=prod[:], in0=src_f32.to_broadcast([128, ncols, 8]),
            in1=MC8[:].rearrange("p (o m) -> p o m", o=1).to_broadcast([128, ncols, 8]),
            op=OP.mult)
        ps1 = psC.tile([16, 8 * ncols], F32, tag="tr")
        nc.tensor.matmul(out=ps1[:], lhsT=W1c[:],
                         rhs=prod[:].rearrange("p c m -> p (c m)"),
                         start=True, stop=True)
        w_sb = tmp.tile([16, 8 * ncols], F32, tag="wr2")
        nc.vector.tensor_copy(out=w_sb[:], in_=ps1[:])
        ps2 = psC.tile([128, 8 * ncols], F32, tag="tr")
        nc.tensor.matmul(out=ps2[:], lhsT=Rrep[:], rhs=w_sb[:],
                         start=True, stop=True)
        nc.vector.tensor_copy(out=dst_i16, in_=ps2[:])

    REPEAT = int(os.environ.get("BASS_REPEAT", "1"))
    for _rep in range(REPEAT):
        _phases(nc, tc, T, pers, big, gat, tmp, one, psA, psB, psC, psG,
                locals())


def _phases(nc, tc, T, pers, big, gat, tmp, one, psA, psB, psC, psG, L):
    AX = mybir.AxisListType
    AX = AX.X
    OP = mybir.AluOpType
    ACTF = mybir.ActivationFunctionType
    import os
    for _k in ("rhs_s lhsT_s pT4 A1c W1t4 A2c W2tf G1 G2 Wv1f ba1r Wv2f ba2r "
               "Wfeatf Wxyzf4 Atd Wu1f Wu2f Wc1f Wc2c bc2T bu_all bcfT W1c MC8 "
               "Rrep iota24 qrow idf32 idf16 ones1 ones128 x1T x2T x3T x4T "
               "idxf_all idx16_all transpose wrap_idx").split():
        globals()["_tmpvar"] = None
    (rhs_s, lhsT_s, pT4, A1c, W1t4, A2c, W2tf, G1, G2, Wv1f, ba1r, Wv2f, ba2r,
     Wfeatf, Wxyzf4, Atd, Wu1f, Wu2f, Wc1f, Wc2c, bc2T, bu_all, bcfT, W1c, MC8,
     Rrep, iota24, qrow, idf32, idf16, ones1, ones128, x1T, x2T, x3T, x4T,
     idxf_all, idx16_all, transpose, wrap_idx) = (
        L["rhs_s"], L["lhsT_s"], L["pT4"], L["A1c"], L["W1t4"], L["A2c"],
        L["W2tf"], L["G1"], L["G2"], L["Wv1f"], L["ba1r"], L["Wv2f"],
        L["ba2r"], L["Wfeatf"], L["Wxyzf4"], L["Atd"], L["Wu1f"], L["Wu2f"],
        L["Wc1f"], L["Wc2c"], L["bc2T"], L["bu_all"], L["bcfT"], L["W1c"],
        L["MC8"], L["Rrep"], L["iota24"], L["qrow"], L["idf32"], L["idf16"],
        L["ones1"], L["ones128"], L["x1T"], L["x2T"], L["x3T"], L["x4T"],
        L["idxf_all"], L["idx16_all"], L["transpose"], L["wrap_idx"])

    # ---------- phase A: T1 shard table + AG1 ----------
    tab1 = pers.tile([128, QB, 128], F16)
    for qb in range(QB):
        ps = psB.tile([128, 128], F32, tag="b")
        nc.tensor.matmul(out=ps[:], lhsT=pT4[:, qb * 128:(qb + 1) * 128],
                         rhs=W1t4[:], start=True, stop=True)
        nc.scalar.copy(out=tab1[:, qb, :], in_=ps[:])
    nc.sync.dma_start(
        out=T["ag1_in"].rearrange("(a p) c -> p a c", p=128),
        in_=tab1[:])
    nc.gpsimd.collective_compute(
        "AllGather", OP.bypass, ins=[T["ag1_in"][:]], outs=[T["ag1_out"][:]],
        replica_groups=T["RG"])

    # ---------- phase B: kNN ----------
    for qb in range(QB):
        s_sb = big.tile([128, N], F32, tag="s_sb")
        m64 = tmp.tile([128, 128], F32, tag="m64")
        for jt in range(16):
            ps = psA.tile([128, 512], F32, tag="sc")
            nc.tensor.matmul(out=ps[:], lhsT=lhsT_s[:, qb * 128:(qb + 1) * 128],
                             rhs=rhs_s[:, jt * 512:(jt + 1) * 512],
                             start=True, stop=True)
            nc.scalar.copy(out=s_sb[:, jt * 512:(jt + 1) * 512], in_=ps[:])
            if jt % 4 == 3:
                h = jt // 4
                nc.vector.tensor_reduce(
                    out=m64[:, h * 32:(h + 1) * 32],
                    in_=s_sb[:, h * 2048:(h + 1) * 2048].rearrange(
                        "p (c e) -> p c e", e=64),
                    axis=AX, op=OP.max)
        nc.sync.dma_start(
            out=T["s_dram"][qb].rearrange("(q c) e -> q (c e)", q=128),
            in_=s_sb[:])
        # top-24 chunks
        cid_u = tmp.tile([128, NCAND], U32, tag="cid_u")
        for r in range(3):
            v8 = tmp.tile([128, 8], F32, tag="v8")
            nc.vector.max(out=v8[:], in_=m64[:])
            nc.vector.max_index(out=cid_u[:, r * 8:(r + 1) * 8], in_max=v8[:],
                                in_values=m64[:])
            nc.vector.match_replace(out=m64[:], in_to_replace=v8[:],
                                    in_values=m64[:], imm_value=NEG)
        cid_f = tmp.tile([128, NCAND], F32, tag="cid_f")
        nc.vector.tensor_copy(out=cid_f[:], in_=cid_u[:])
        crow = tmp.tile([128, NCAND], F32, tag="crow")
        nc.vector.tensor_scalar(out=crow[:], in0=cid_f[:], scalar1=qrow[:, 0:1],
                                scalar2=None, op0=OP.add)
        cidx16 = tmp.tile([128, 8 * NCAND], I16, tag="cidx16")
        wrap_idx(crow[:], NCAND, cidx16[:])
        cand = gat.tile([128, NCAND, 64], F32, tag="cand")
        nc.gpsimd.dma_gather(out_ap=cand[:], in_ap=T["s_dram"][qb][:],
                             idxs_ap=cidx16[:], num_idxs=128 * NCAND,
                             num_idxs_reg=128 * NCAND, elem_size=64,
                             single_packet=False)
        # top-16 among candidates
        pos_u = tmp.tile([128, 16], U32, tag="pos_u")
        cand_f = cand[:].rearrange("p c e -> p (c e)")
        for r in range(2):
            v8 = tmp.tile([128, 8], F32, tag="v8")
            nc.vector.max(out=v8[:], in_=cand_f)
            nc.vector.max_index(out=pos_u[:, r * 8:(r + 1) * 8], in_max=v8[:],
                                in_values=cand_f)
            if r == 0:
                nc.vector.match_replace(out=cand_f, in_to_replace=v8[:],
                                        in_values=cand_f, imm_value=NEG)
        # j = cid[pos//64]*64 + pos%64
        csl_u = tmp.tile([128, 16], U32, tag="csl_u")
        nc.vector.tensor_scalar(out=csl_u[:], in0=pos_u[:], scalar1=6,
                                scalar2=None, op0=OP.logical_shift_right)
        rem_u = tmp.tile([128, 16], U32, tag="rem_u")
        nc.vector.tensor_scalar(out=rem_u[:], in0=pos_u[:], scalar1=63,
                                scalar2=None, op0=OP.bitwise_and)
        csf = tmp.tile([128, 16], F32, tag="csf")
        nc.vector.tensor_copy(out=csf[:], in_=csl_u[:])
        remf = tmp.tile([128, 16], F32, tag="remf")
        nc.vector.tensor_copy(out=remf[:], in_=rem_u[:])
        eq = tmp.tile([128, NCAND, 16], F32, tag="eq")
        nc.vector.tensor_tensor(
            out=eq[:], in0=csf[:].rearrange("p (o k) -> p o k", o=1).to_broadcast([128, NCAND, 16]),
            in1=iota24[:].to_broadcast([128, NCAND, 16]), op=OP.is_equal)
        nc.vector.tensor_tensor(
            out=eq[:], in0=eq[:],
            in1=cid_f[:].to_broadcast([128, NCAND, 16]), op=OP.mult)
        csel = tmp.tile([128, 16], F32, tag="csel")
        nc.vector.tensor_reduce(
            out=csel[:], in_=eq[:].rearrange("p c k -> p k c"),
            axis=AX, op=OP.add)
        nc.vector.tensor_scalar(out=csel[:], in0=csel[:], scalar1=64.0,
                                scalar2=None, op0=OP.mult)
        nc.vector.tensor_tensor(out=idxf_all[:, qb, :], in0=csel[:],
                                in1=remf[:], op=OP.add)
        nc.sync.dma_start(out=T["idx_dram"][qb * 128:(qb + 1) * 128, :],
                            in_=idxf_all[:, qb, :])
        wrap_idx(idxf_all[:, qb, :], 16, idx16_all[:, qb, :])

    import os
    nc.sync.dma_start(out=T["dbg"][:], in_=idxf_all[:])
    if os.environ.get("PHASES", "ALL") == "B":
        nc.sync.dma_start(out=T["out_t"][:], in_=oT_dummy(nc, one)[:])
        return

    # ---------- generic edge-conv stage ----------
    def layer_gathers(table_ap, C, qb_per):
        """Consolidated gathers: QB//qb_per dma_gathers over qb groups."""
        tiles = []
        for h in range(QB // qb_per):
            gb = gat.tile([128, qb_per * 16, C], F16, tag="g")
            nc.gpsimd.dma_gather(
                out_ap=gb[:], in_ap=table_ap,
                idxs_ap=idx16_all[:, h * qb_per:(h + 1) * qb_per, :].rearrange(
                    "p a c -> p (a c)"),
                num_idxs=qb_per * 2048, num_idxs_reg=qb_per * 2048,
                elem_size=C, single_packet=False)
            tiles.append(gb)

        def view(qb):
            return tiles[qb // qb_per][
                :, (qb % qb_per) * 16:((qb % qb_per) + 1) * 16, :]
        return view

    # ---------- phase C: EdgeConv1 (3 -> 64) ----------
    gv1 = layer_gathers(T["ag1_out"][:], 128, 4)
    for qb in range(QB):
        g = gv1(qb)
        mx = tmp.tile([128, 64], F32, tag="mx1")
        nc.vector.tensor_reduce(
            out=mx[:], in_=g[:, :, 0:64].rearrange("p k c -> p c k"),
            axis=AX, op=OP.max)
        ps = psB.tile([128, 64], F32, tag="b")
        nc.tensor.matmul(out=ps[:], lhsT=pT4[:, qb * 128:(qb + 1) * 128],
                         rhs=A1c[:], start=True, stop=True)
        nc.vector.tensor_tensor(out=mx[:], in0=mx[:], in1=ps[:], op=OP.add)
        x1q = tmp.tile([128, 64], F16, tag="x1q")
        nc.vector.tensor_scalar(out=x1q[:], in0=mx[:], scalar1=0.0,
                                scalar2=None, op0=OP.max)
        tr = transpose(x1q[:], 128, 64, F16)
        nc.vector.tensor_copy(out=x1T[0:64, qb * 128:(qb + 1) * 128],
                              in_=tr[:, :128])

    # ---------- phase D: T2 shard + AG2 ----------
    tab2 = pers.tile([128, QB, 128], F16)
    for qb in range(QB):
        ps = psB.tile([128, 128], F32, tag="b")
        nc.tensor.matmul(out=ps[:], lhsT=x1T[0:64, qb * 128:(qb + 1) * 128],
                         rhs=W2tf[:], start=True, stop=True)
        nc.scalar.copy(out=tab2[:, qb, :], in_=ps[:])
    nc.sync.dma_start(
        out=T["ag2_in"].rearrange("(a p) c -> p a c", p=128), in_=tab2[:])
    nc.gpsimd.collective_compute(
        "AllGather", OP.bypass, ins=[T["ag2_in"][:]], outs=[T["ag2_out"][:]],
        replica_groups=T["RG"])

    # ---------- phase E: EdgeConv2 (64 -> 128) + x2 AG ----------
    gv2 = layer_gathers(T["ag2_out"][:], 128, 4)
    for qb in range(QB):
        g = gv2(qb)
        mx = tmp.tile([128, 128], F32, tag="mx2")
        nc.vector.tensor_reduce(
            out=mx[:], in_=g[:].rearrange("p k c -> p c k"), axis=AX, op=OP.max)
        ps = psB.tile([128, 128], F32, tag="b")
        nc.tensor.matmul(out=ps[:], lhsT=x1T[:, qb * 128:(qb + 1) * 128],
                         rhs=A2c[:], start=True, stop=True)
        nc.vector.tensor_tensor(out=mx[:], in0=mx[:], in1=ps[:], op=OP.add)
        x2q = tmp.tile([128, 128], F16, tag="x2q")
        nc.vector.tensor_scalar(out=x2q[:], in0=mx[:], scalar1=0.0,
                                scalar2=None, op0=OP.max)
        tr = transpose(x2q[:], 128, 128, F16)
        nc.vector.tensor_copy(out=x2T[:, qb * 128:(qb + 1) * 128], in_=tr[:])
        nc.sync.dma_start(out=T["ag3_in"][qb * 128:(qb + 1) * 128, :],
                            in_=x2q[:])
    nc.gpsimd.collective_compute(
        "AllGather", OP.bypass, ins=[T["ag3_in"][:]], outs=[T["ag3_out"][:]],
        replica_groups=T["RG"])

    # ---------- attention stage ----------
    def attention(qb, g, D, u_tile, zT_out):
        """Gathered x_j rows g [128,16,D], logits via ttr with u, softmax, z."""
        l = tmp.tile([128, 16], F32, tag="l")
        prod = one.tile([128, 16, D], F16, tag="prod")
        nc.vector.tensor_tensor(
            out=prod[:], in0=g[:],
            in1=u_tile.rearrange("p (o c) -> p o c", o=1).to_broadcast([128, 16, D]),
            op=OP.mult)
        nc.vector.tensor_reduce(out=l[:], in_=prod[:], axis=AX, op=OP.add)
        mx = tmp.tile([128, 1], F32, tag="lm")
        nc.vector.tensor_reduce(out=mx[:], in_=l[:], axis=AX, op=OP.max)
        nmx = tmp.tile([128, 1], F32, tag="nmx")
        nc.vector.tensor_scalar(out=nmx[:], in0=mx[:], scalar1=-1.0,
                                scalar2=None, op0=OP.mult)
        a = tmp.tile([128, 16], F32, tag="a")
        ssum = tmp.tile([128, 1], F32, tag="ssum")
        nc.scalar.activation(out=a[:], in_=l[:], func=ACTF.Exp,
                             bias=nmx[:, 0:1], accum_out=ssum[:])
        for k in range(16):
            nc.scalar.activation(out=g[:, k, :], in_=g[:, k, :],
                                 func=ACTF.Copy, scale=a[:, k:k + 1])
        nc.vector.tensor_tensor(out=g[:, 0:8, :], in0=g[:, 0:8, :],
                                in1=g[:, 8:16, :], op=OP.add)
        nc.vector.tensor_tensor(out=g[:, 0:4, :], in0=g[:, 0:4, :],
                                in1=g[:, 4:8, :], op=OP.add)
        nc.vector.tensor_tensor(out=g[:, 0:2, :], in0=g[:, 0:2, :],
                                in1=g[:, 2:4, :], op=OP.add)
        zf = tmp.tile([128, D], F16, tag=f"zf{D}")
        nc.vector.tensor_tensor(out=zf[:], in0=g[:, 0, :],
                                in1=g[:, 1, :], op=OP.add)
        rcp = tmp.tile([128, 1], F32, tag="rcp")
        nc.vector.reciprocal(out=rcp[:], in_=ssum[:])
        nc.vector.tensor_scalar(out=zf[:], in0=zf[:], scalar1=rcp[:, 0:1],
                                scalar2=None, op0=OP.mult)
        for c in range(D // 128):
            tr = transpose(zf[:, c * 128:(c + 1) * 128], 128, 128, F16)
            nc.vector.tensor_copy(out=zT_out[:, c, :], in_=tr[:])

    if os.environ.get("PHASES", "ALL") == "E":
        nc.sync.dma_start(out=T["out_t"][:], in_=oT_dummy(nc, one)[:])
        return

    # ---------- phase F: attention 1 (128 -> 256) + x3/AG4 ----------
    gv3 = layer_gathers(T["ag3_out"][:], 128, 1)
    for qb in range(QB):
        psu = psB.tile([128, 128], F32, tag="b")
        nc.tensor.matmul(out=psu[:], lhsT=x2T[:, qb * 128:(qb + 1) * 128],
                         rhs=G1[:], start=True, stop=True)
        u1 = tmp.tile([128, 128], F16, tag="u1")
        nc.scalar.copy(out=u1[:], in_=psu[:])
        z1T = tmp.tile([128, 1, 128], F16, tag="z1T")
        attention(qb, gv3(qb), 128, u1[:], z1T[:])
        psy = psB.tile([128, 256], F32, tag="b")
        nc.tensor.matmul(out=psy[:], lhsT=z1T[:, 0, :], rhs=Wv1f[:],
                         start=True, stop=False)
        nc.tensor.matmul(out=psy[:], lhsT=ones1[:], rhs=ba1r[:],
                         start=False, stop=True)
        x3q = tmp.tile([128, 256], F16, tag="x3q")
        nc.vector.tensor_scalar(out=x3q[:], in0=psy[:], scalar1=0.0,
                                scalar2=None, op0=OP.max)
        for c in range(2):
            tr = transpose(x3q[:, c * 128:(c + 1) * 128], 128, 128, F16)
            nc.vector.tensor_copy(out=x3T[:, c, qb * 128:(qb + 1) * 128],
                                  in_=tr[:])
        nc.sync.dma_start(out=T["ag4_in"][qb * 128:(qb + 1) * 128, :],
                            in_=x3q[:])
    nc.gpsimd.collective_compute(
        "AllGather", OP.bypass, ins=[T["ag4_in"][:]], outs=[T["ag4_out"][:]],
        replica_groups=T["RG"])

    if os.environ.get("PHASES", "ALL") == "F":
        nc.sync.dma_start(out=T["out_t"][:], in_=oT_dummy(nc, one)[:])
        return

    # ---------- phase G: attention 2 (256 -> 512) ----------
    gv4 = layer_gathers(T["ag4_out"][:], 256, 1)
    for qb in range(QB):
        psu = psB.tile([128, 256], F32, tag="b")
        for c in range(2):
            nc.tensor.matmul(out=psu[:], lhsT=x3T[:, c, qb * 128:(qb + 1) * 128],
                             rhs=G2[:, c, :], start=(c == 0), stop=(c == 1))
        u2 = tmp.tile([128, 256], F16, tag="u2")
        nc.scalar.copy(out=u2[:], in_=psu[:])
        z2T = tmp.tile([128, 2, 128], F16, tag="z2T")
        attention(qb, gv4(qb), 256, u2[:], z2T[:])
        psy = psB.tile([128, 512], F32, tag="b")
        for c in range(2):
            nc.tensor.matmul(out=psy[:], lhsT=z2T[:, c, :], rhs=Wv2f[:, c, :],
                             start=(c == 0), stop=False)
        nc.tensor.matmul(out=psy[:], lhsT=ones1[:], rhs=ba2r[:],
                         start=False, stop=True)
        x4q = tmp.tile([128, 512], F16, tag="x4q")
        nc.vector.tensor_scalar(out=x4q[:], in0=psy[:], scalar1=0.0,
                                scalar2=None, op0=OP.max)
        for c in range(4):
            tr = transpose(x4q[:, c * 128:(c + 1) * 128], 128, 128, F16)
            nc.vector.tensor_copy(out=x4T[:, c, qb * 128:(qb + 1) * 128],
                                  in_=tr[:])

    # ---------- phase H: TD table + AG5 ----------
    tabT = pers.tile([128, QB, 512], F16)
    for qb in range(QB):
        ps = psB.tile([128, 512], F32, tag="b")
        for c in range(4):
            nc.tensor.matmul(out=ps[:], lhsT=x4T[:, c, qb * 128:(qb + 1) * 128],
                             rhs=Wfeatf[:, c, :], start=(c == 0), stop=False)
        nc.tensor.matmul(out=ps[:], lhsT=pT4[:, qb * 128:(qb + 1) * 128],
                         rhs=Wxyzf4[:], start=False, stop=True)
        nc.scalar.copy(out=tabT[:, qb, :], in_=ps[:])
    nc.sync.dma_start(
        out=T["ag5_in"].rearrange("(a p) c -> p a c", p=128), in_=tabT[:])
    nc.gpsimd.collective_compute(
        "AllGather", OP.bypass, ins=[T["ag5_in"][:]], outs=[T["ag5_out"][:]],
        replica_groups=T["RG"])

    if os.environ.get("PHASES", "ALL") == "H":
        nc.sync.dma_start(out=T["out_t"][:], in_=oT_dummy(nc, one)[:])
        return

    # ---------- phase I: TransitionDown ----------
    xdT = pers.tile([128, 4, MQ], F16)
    psg = psG.tile([1, 512], F32, tag="gsum")
    idxd16_all = one.tile([128, TDQB, 128], I16, tag="idxd16")
    for t in range(TDQB):
        idxd = one.tile([128, 16], F32, tag=f"idxd{t}")
        nc.sync.dma_start(
            out=idxd[:],
            in_=T["idx_dram"][t * 512:(t + 1) * 512:4, :])
        wrap_idx(idxd[:], 16, idxd16_all[:, t, :])
    gtds = []
    for t in range(TDQB):
        gtd = gat.tile([128, 16, 512], F16, tag="g")
        nc.gpsimd.dma_gather(out_ap=gtd[:], in_ap=T["ag5_out"][:],
                             idxs_ap=idxd16_all[:, t, :], num_idxs=2048,
                             num_idxs_reg=2048, elem_size=512,
                             single_packet=False)
        gtds.append(gtd)
    for t in range(TDQB):
        g = gtds[t]
        mx = one.tile([128, 512], F32, tag="mxtd")
        nc.vector.tensor_reduce(
            out=mx[:], in_=g[:].rearrange("p k c -> p c k"), axis=AX, op=OP.max)
        ps = psB.tile([128, 512], F32, tag="b")
        nc.tensor.matmul(out=ps[:], lhsT=pT4[:, t * 512::4][:, :128],
                         rhs=Atd[:], start=True, stop=True)
        xd = one.tile([128, 512], F32, tag="xd")
        nc.vector.tensor_tensor(out=xd[:], in0=mx[:], in1=ps[:], op=OP.add)
        nc.vector.tensor_scalar(out=xd[:], in0=xd[:], scalar1=0.0,
                                scalar2=None, op0=OP.max)
        nc.tensor.matmul(out=psg[:], lhsT=ones128[:], rhs=xd[:],
                         start=(t == 0), stop=(t == TDQB - 1))
        xdf = one.tile([128, 512], F16, tag="xdf")
        nc.vector.tensor_copy(out=xdf[:], in_=xd[:])
        for c in range(4):
            tr = transpose(xdf[:, c * 128:(c + 1) * 128], 128, 128, F16)
            nc.vector.tensor_copy(out=xdT[:, c, t * 128:(t + 1) * 128],
                                  in_=tr[:])
    gsum = one.tile([1, 512], F32, tag="gsumsb")
    nc.vector.tensor_copy(out=gsum[:], in_=psg[:])
    nc.sync.dma_start(out=T["ar_in"][:], in_=gsum[:])
    nc.gpsimd.collective_compute(
        "AllReduce", OP.add, ins=[T["ar_in"][:]], outs=[T["ar_out"][:]],
        replica_groups=T["RG"])

    # ---------- phase J: TransitionUp + classifier ----------
    gs = one.tile([1, 512], F32, tag="gs")
    nc.sync.dma_start(out=gs[:], in_=T["ar_out"][:])
    gm = one.tile([1, 512], F32, tag="gm")
    nc.scalar.mul(out=gm[:], in_=gs[:], mul=1.0 / 2048.0)
    gmT = one.tile([128, 4], F32, tag="gmT")
    for c in range(4):
        tr = transpose(gm[0:1, c * 128:(c + 1) * 128], 1, 128, F32)
        nc.vector.tensor_copy(out=gmT[:, c:c + 1], in_=tr[:, 0:1])
    gbT = one.tile([128, 2], F32, tag="gbT")
    for cc in range(2):
        ps = psC.tile([128, 1], F32, tag="tr")
        for c in range(4):
            nc.tensor.matmul(out=ps[:], lhsT=Wu2f[:, c, cc * 128:(cc + 1) * 128],
                             rhs=gmT[:, c:c + 1], start=(c == 0), stop=(c == 3))
        nc.vector.tensor_copy(out=gbT[:, cc:cc + 1], in_=ps[:])
    nc.vector.tensor_tensor(out=gbT[:], in0=gbT[:], in1=bu_all[:], op=OP.add)

    xuT = one.tile([128, 2, MQ], F16, tag="xuT")
    for cc in range(2):
        ps = psB.tile([128, MQ], F32, tag="b")
        for c in range(4):
            nc.tensor.matmul(out=ps[:], lhsT=Wu1f[:, c, cc * 128:(cc + 1) * 128],
                             rhs=xdT[:, c, :], start=(c == 0), stop=(c == 3))
        nc.scalar.activation(out=xuT[:, cc, :], in_=ps[:], func=ACTF.Relu,
                             bias=gbT[:, cc:cc + 1], scale=1.0)
    psh = psB.tile([128, MQ], F32, tag="b")
    for c in range(2):
        nc.tensor.matmul(out=psh[:], lhsT=Wc1f[:, c, :], rhs=xuT[:, c, :],
                         start=(c == 0), stop=(c == 1))
    hT = one.tile([128, MQ], F16, tag="hT")
    nc.scalar.activation(out=hT[:], in_=psh[:], func=ACTF.Relu,
                         bias=bcfT[:, 0:1], scale=1.0)
    pso = psC.tile([6, MQ], F32, tag="tr")
    nc.tensor.matmul(out=pso[:], lhsT=Wc2c[:], rhs=hT[:], start=True, stop=True)
    oT = one.tile([6, MQ], F32, tag="oT")
    nc.scalar.add(out=oT[:], in_=pso[:], add=bc2T[:, 0:1])
    nc.sync.dma_start(out=T["out_t"][:], in_=oT[:])


def _prepare_inputs(inputs):
    p = np.asarray(inputs["points"])[0].astype(np.float32)        # [N, 3]
    pp = (p.astype(np.float64) ** 2).sum(1).astype(np.float32)

    qh, qm, ql = _split3(2.0 * p.T)                   # [3, N]
    ph, pm_, pl = _split3(p.T)
    pph, ppm, ppl = _split3(-pp[None, :])
    onesN = np.ones((1, N), np.float32)
    lhsTs_full = np.concatenate([qh, qh, qh, qm, qm, ql, onesN, onesN, onesN],
                                0).astype(NPBF)       # [21, N]
    rhss = np.concatenate([ph, pm_, pl, ph, pm_, ph, pph, ppm, ppl],
                          0).astype(NPBF)             # [21, N]
    pT4_full = np.concatenate([p.T, onesN], 0).astype(np.float32)  # [4, N]

    f32 = np.float32
    We1, se1, be1 = f32(inputs["We1"]), f32(inputs["se1"]), f32(inputs["be1"])
    We2, se2, be2 = f32(inputs["We2"]), f32(inputs["se2"]), f32(inputs["be2"])
    Wq1, Wk1, Wv1 = f32(inputs["Wq1"]), f32(inputs["Wk1"]), f32(inputs["Wv1"])
    sa1, ba1 = f32(inputs["sa1"]), f32(inputs["ba1"])
    Wq2, Wk2, Wv2 = f32(inputs["Wq2"]), f32(inputs["Wk2"]), f32(inputs["Wv2"])
    sa2, ba2 = f32(inputs["sa2"]), f32(inputs["ba2"])
    Wtd, sd, bd = f32(inputs["Wtd"]), f32(inputs["sd"]), f32(inputs["bd"])
    Wu1, su1, bu1 = f32(inputs["Wu1"]), f32(inputs["su1"]), f32(inputs["bu1"])
    Wu2, su2, bu2 = f32(inputs["Wu2"]), f32(inputs["su2"]), f32(inputs["bu2"])
    Wc1, bc1 = f32(inputs["Wc1"]), f32(inputs["bc1"])
    sc, bc = f32(inputs["sc"]), f32(inputs["bc"])
    Wc2, bc2 = f32(inputs["Wc2"]), f32(inputs["bc2"])

    A1c = np.concatenate([(We1[:3] - We1[3:]) * se1, (be1)[None, :]], 0)
    W1t4 = np.zeros((4, 128), np.float32)
    W1t4[:3, :64] = We1[3:] * se1
    A2c = np.concatenate([(We2[:64] - We2[64:]) * se2, be2[None, :]],
                         0).astype(NPF16)
    W2tf = (We2[64:] * se2).astype(NPF16)
    G1 = (Wq1 @ Wk1.T / np.sqrt(256.0)).astype(NPF16)
    G2 = (Wq2 @ Wk2.T / np.sqrt(512.0)).astype(NPF16)
    Wv1f = (Wv1 * sa1).astype(NPF16)
    Wv2f = (Wv2 * sa2).astype(NPF16)
    Wxyz = Wtd[:3] * sd
    Wxyzf4 = np.concatenate([Wxyz, np.zeros((1, 512), np.float32)], 0)
    Atd = np.concatenate([-Wxyz, bd[None, :]], 0)
    Wfeatf = (Wtd[3:] * sd).astype(NPF16)
    Wu1f = (Wu1 * su1).astype(NPF16)
    Wu2f = (Wu2 * su2).astype(np.float32)
    Wc1f = (Wc1 * sc).astype(NPF16)
    bcf = (bc1 * sc + bc).astype(np.float32)

    qi = np.arange(128)
    W1c = (qi[:, None] % 16 == np.arange(16)[None, :]).astype(np.float32)
    MC8 = (qi[:, None] // 16 == np.arange(8)[None, :]).astype(np.float32)
    Rrep = (np.arange(16)[:, None] == qi[None, :] % 16).astype(np.float32)
    iota24 = np.tile(np.arange(NCAND, dtype=np.float32)[None, :], (128, 1))
    qrow = (qi * 128).astype(np.float32)[:, None]

    common = dict(
        rhs_s=rhss, A1c=A1c, W1t4=W1t4, A2c=A2c, W2tf=W2tf, G1=G1, G2=G2,
        Wv1f=Wv1f, ba1r=(ba1[None, :]).astype(NPF16),
        Wv2f=Wv2f, ba2r=(ba2[None, :]).astype(NPF16),
        Wfeatf=Wfeatf, Wxyzf4=Wxyzf4, Atd=Atd,
        Wu1f=Wu1f, Wu2f=Wu2f, Wc1f=Wc1f,
        Wc2c=Wc2.astype(NPF16), bc2T=bc2.astype(np.float32)[:, None],
        bu_all=(bu1 + bu2).astype(np.float32)[:, None],
        bcfT=bcf[:, None],
        W1c=W1c, MC8=MC8, Rrep=Rrep, iota24=iota24, qrow=qrow,
    )
    in_maps = []
    for c in range(NCORES):
        m = dict(common)
        m["lhsT_s"] = np.ascontiguousarray(lhsTs_full[:, c * NQ:(c + 1) * NQ])
        m["pT4"] = np.ascontiguousarray(pT4_full[:, c * NQ:(c + 1) * NQ])
        in_maps.append(m)
    return in_maps


def kernel(**inputs):
    if "nc" not in _cache:
        _cache["nc"] = _build_program()
    nc = _cache["nc"]
    in_maps = _prepare_inputs(inputs)
    res = run_bass_kernel_spmd(nc, in_maps, core_ids=list(range(NCORES)))
    out = np.zeros((2048, 6), np.float32)
    for c in range(NCORES):
        out[c * MQ:(c + 1) * MQ, :] = res.results[c]["outT"].T
    _cache["dbg"] = [res.results[c]["dbg"] for c in range(NCORES)]
    _cache["res"] = res
    return out

